# revision 4
# baseline (speedup 1.0000x reference)
"""BiLSTM-CRF on 8 Trainium2 NeuronCores — batch-data-parallel, v2.

Contract: kernel(**inputs) takes the FULL unsharded inputs (as produced by
setup_inputs) and returns the full [B, T] int32 tag tensor.

v2 changes vs baseline:
  - embedding gather + input transpose moved to host (x0T shipped per core)
  - optional hi/lo weight compensation per weight group (flags below)
  - recurrence: per-bank PSUM accumulation (sigmoid starts after half the
    matmuls), c/sigma(o) transposed instead of h (shorter critical chain,
    single fused write of hT into the next layer's input tile)
  - Viterbi backtrace on device via one-hot selection matmuls; outputs are
    the final [4, T] tag rows per chain (130x smaller transfers)
"""

import numpy as np

B, T_FULL, V, E, H, K = 64, 512, 30000, 256, 256, 32
NCORES = 8
BS = B // NCORES
G4 = 4 * H

WCOMP_IH = False  # host hi/lo split of W_ih (proj weights)
WCOMP_HH = False  # host hi/lo split of W_hh (recurrent weights)
WCOMP_OUT = True  # host hi/lo split of W_out (emissions; cheap, keep)

_cache = {}


def _split_hi_lo(w):
    w = np.asarray(w, np.float32)
    m, e = np.frexp(w)
    hi = np.ldexp(np.round(m * 4096.0) / 4096.0, e).astype(np.float32)
    lo = (w.astype(np.float64) - hi).astype(np.float32)
    return hi, lo


def _pack_gates_rows(w):
    """Reorder leading 4H axis from (i,f,g,o) to (i,f,o,g)."""
    i, f, g, o = np.split(np.asarray(w), 4, axis=0)
    return np.concatenate([i, f, o, g], axis=0)


def _build(T_):
    import concourse.bass as bass
    import concourse.tile as tile
    from concourse import bacc, mybir
    from concourse.masks import make_identity

    f32 = mybir.dt.float32
    f32r = mybir.dt.float32r
    u16 = mybir.dt.uint16
    u32 = mybir.dt.uint32
    i32 = mybir.dt.int32
    AF = mybir.ActivationFunctionType
    OP = mybir.AluOpType
    AP = bass.AP

    ntok = BS * T_
    nw128 = ntok // 128
    NWI = 2 if WCOMP_IH else 1
    NWH = 2 if WCOMP_HH else 1
    NWO = 2 if WCOMP_OUT else 1

    nc = bacc.Bacc(None, target_bir_lowering=False)

    x0T_d = nc.dram_tensor("x0T", [2 * 128, ntok], f32r, kind="ExternalInput")
    wih0_d = nc.dram_tensor("wih0", [2 * 2 * NWI * 128, G4], f32r, kind="ExternalInput")
    wih1_d = nc.dram_tensor("wih1", [2 * 4 * NWI * 128, G4], f32r, kind="ExternalInput")
    whh0_d = nc.dram_tensor("whh0", [2 * 2 * NWH * 128, G4], f32r, kind="ExternalInput")
    whh1_d = nc.dram_tensor("whh1", [2 * 2 * NWH * 128, G4], f32r, kind="ExternalInput")
    brow_d = nc.dram_tensor("brow", [1, 4 * G4], f32r, kind="ExternalInput")
    wout_d = nc.dram_tensor("wout", [4 * NWO * 128, K], f32r, kind="ExternalInput")
    bout_d = nc.dram_tensor("bout", [K, 1], f32, kind="ExternalInput")
    ssel_d = nc.dram_tensor("ssel", [128, 128], f32r, kind="ExternalInput")
    ones_d = nc.dram_tensor("ones", [1, 128], f32r, kind="ExternalInput")
    trep_d = nc.dram_tensor("trep", [128, K], f32, kind="ExternalInput")
    srep_d = nc.dram_tensor("srep", [128, 1], f32, kind="ExternalInput")
    erep_d = nc.dram_tensor("erep", [128, 1], f32, kind="ExternalInput")
    kidx_d = nc.dram_tensor("kidx", [128, 4], f32r, kind="ExternalInput")
    sel4_d = nc.dram_tensor("sel4", [128, 4], f32r, kind="ExternalInput")

    tags_d = [nc.dram_tensor(f"tags{c}", [1, 4 * T_], i32, kind="ExternalOutput") for c in range(2)]

    gx_d = {(l, d): nc.dram_tensor(f"gx{l}{d}", [ntok, G4], f32r, kind="Internal")
            for l in range(2) for d in range(2)}
    em_d = nc.dram_tensor("em_scr", [K, ntok], f32, kind="Internal")

    with tile.TileContext(nc) as tc:
        with (
            tc.tile_pool(name="const", bufs=1) as cpool,
        ):
            whh = {}
            for l, dram in [(0, whh0_d), (1, whh1_d)]:
                for d in range(2):
                    for k in range(2 * NWH):
                        t_ = cpool.tile([128, G4], f32r, tag=f"whh{l}{d}{k}", name=f"whh{l}{d}{k}")
                        nc.sync.dma_start(t_[:], dram[(d * 2 * NWH + k) * 128:(d * 2 * NWH + k + 1) * 128, :])
                        whh[(l, d, k)] = t_
            ssel = cpool.tile([128, 128], f32r, tag="ssel")
            nc.sync.dma_start(ssel[:], ssel_d[:])
            brow = cpool.tile([1, 4 * G4], f32r, tag="brow")
            nc.sync.dma_start(brow[:], brow_d[:])
            ones1 = cpool.tile([1, 128], f32r, tag="ones1")
            nc.sync.dma_start(ones1[:], ones_d[:])
            id8 = cpool.tile([8, 8], f32, tag="id8")
            make_identity(nc, id8[:])
            wout = {}
            for k in range(4 * NWO):
                t_ = cpool.tile([128, K], f32r, tag=f"wout{k}", name=f"wout{k}")
                nc.sync.dma_start(t_[:], wout_d[k * 128:(k + 1) * 128, :])
                wout[k] = t_
            bout = cpool.tile([K, 1], f32, tag="bout")
            nc.sync.dma_start(bout[:], bout_d[:])
            trep = cpool.tile([128, K], f32, tag="trep")
            nc.sync.dma_start(trep[:], trep_d[:])
            srep = cpool.tile([128, 1], f32, tag="srep")
            nc.sync.dma_start(srep[:], srep_d[:])
            erep = cpool.tile([128, 1], f32, tag="erep")
            nc.sync.dma_start(erep[:], erep_d[:])
            kidx = cpool.tile([128, 4], f32r, tag="kidx")
            nc.sync.dma_start(kidx[:], kidx_d[:])
            sel4 = cpool.tile([128, 4], f32r, tag="sel4")
            nc.sync.dma_start(sel4[:], sel4_d[:])

            def proj_phase(l, xT_view, nkc):
                """xT_view(k, lo, hi) -> AP [128, hi-lo] of input chunk k."""
                wih_dram = wih0_d if l == 0 else wih1_d
                with (
                    tc.tile_pool(name=f"proj{l}", bufs=1) as pp,
                    tc.tile_pool(name=f"projps{l}", bufs=2, space="PSUM") as ppp,
                ):
                    for d in range(2):
                        wih = {}
                        for k in range(nkc * NWI):
                            t_ = pp.tile([128, G4], f32r, tag=f"wih{k}", name=f"wih{k}")
                            nc.sync.dma_start(
                                t_[:],
                                wih_dram[(d * nkc * NWI + k) * 128:(d * nkc * NWI + k + 1) * 128, :],
                            )
                            wih[k] = t_
                        for w in range(nw128):
                            ps = ppp.tile([128, G4], f32, tag="pj")
                            for nh in range(2):
                                sl = slice(nh * 512, (nh + 1) * 512)
                                nc.tensor.matmul(
                                    ps[:, sl], ones1[:, :],
                                    brow[0:1, (l * 2 + d) * G4 + nh * 512:(l * 2 + d) * G4 + (nh + 1) * 512],
                                    start=True, stop=False,
                                )
                                for k in range(nkc * NWI):
                                    nc.tensor.matmul(
                                        ps[:, sl],
                                        xT_view(k // NWI, w * 128, (w + 1) * 128),
                                        wih[k][:, sl],
                                        start=False, stop=(k == nkc * NWI - 1),
                                    )
                            st = pp.tile([128, G4], f32r, tag="pjst", bufs=2)
                            nc.vector.tensor_copy(st[:], ps[:])
                            nc.sync.dma_start(gx_d[(l, d)][w * 128:(w + 1) * 128, :], st[:])

            def rec_phase(l, xnext):
                """xnext: dict d -> tile [128, 2*ntok] (k-chunk major)."""
                with (
                    tc.tile_pool(name=f"rec{l}", bufs=1) as rp,
                    tc.tile_pool(name=f"recps{l}", bufs=1, space="PSUM") as rpp,
                ):
                    # tgc: [tanh(g) | c] adjacent so t1|t2 is ONE tensor_tensor
                    tgc = {d: rp.tile([8, 512], f32, tag=f"tgc{d}", name=f"tgc{d}") for d in range(2)}
                    nwin = T_ // 16

                    def win_dma(d, wi):
                        wt = rp.tile([128, G4], f32r, tag=f"win{d}", bufs=2, name=f"win{d}")
                        nc.sync.dma_start(
                            wt[:],
                            AP(gx_d[(l, d)], wi * 16 * G4,
                               [[T_ * G4, BS], [G4, 16], [1, G4]]),
                        )
                        return wt

                    win = {0: win_dma(0, 0), 1: win_dma(1, nwin - 1)}
                    win_next = {}
                    xv = {d: xnext[d][:].rearrange("p (k b t) -> p k t b", k=2, b=BS)
                          for d in range(2)}

                    def sj_of(t, d):
                        s = t if d == 0 else T_ - 1 - t
                        return s, s % 16

                    def emit_inject_d(d, t):
                        """Window bookkeeping + gx-inject matmuls for (d, t)."""
                        s, j = sj_of(t, d)
                        wi = s // 16
                        if t > 0 and j == (0 if d == 0 else 15):
                            win[d] = win_next[d]
                        if j == (8 if d == 0 else 7):
                            nwi = wi + 1 if d == 0 else wi - 1
                            if 0 <= nwi < nwin:
                                win_next[d] = win_dma(d, nwi)
                        gbs = {}
                        for nh in range(2):
                            g_ps = rpp.tile([8, 512], f32, tag=f"g{d}{nh}",
                                            bufs=(2 if d == 0 else 1), name=f"g{d}{nh}")
                            nc.tensor.matmul(
                                g_ps[:], ssel[:, j * 8:(j + 1) * 8],
                                win[d][:, nh * 512:(nh + 1) * 512],
                                start=True, stop=(t == 0),
                            )
                            gbs[nh] = g_ps
                        return gbs

                    # The two direction-chains run skewed by one step: while d0
                    # is at step t, d1 is at step t-1, so each dir's ACT block
                    # fills the other's non-ACT spine segments.
                    gb = {0: emit_inject_d(0, 0), 1: None}
                    sif = [None, None]
                    so = [None, None]
                    t12 = [None, None]
                    tp = [None, None]
                    thT = [None, None]

                    def emit_rec_mms(d, t):
                        if t == 0:
                            return
                        s, j = sj_of(t, d)
                        sp = s - 1 if d == 0 else s + 1
                        for nh in range(2):
                            for k in range(2 * NWH):
                                nc.tensor.matmul(
                                    gb[d][nh][:],
                                    xv[d][:, k // NWH, sp, :],
                                    whh[(l, d, k)][:, nh * 512:(nh + 1) * 512],
                                    start=False,
                                    stop=(k == 2 * NWH - 1),
                                )

                    def emit_sig(d, t):
                        sif[d] = rp.tile([8, 512], f32, tag=f"sif{d}", bufs=2, name=f"sif{d}")
                        nc.scalar.activation(sif[d][:], gb[d][0][:], AF.Sigmoid)
                        nc.scalar.activation(tgc[d][:, 0:H], gb[d][1][:, H:2 * H], AF.Tanh)
                        so[d] = rp.tile([8, H], f32, tag=f"so{d}", bufs=2, name=f"so{d}")
                        nc.scalar.activation(so[d][:], gb[d][1][:, 0:H], AF.Sigmoid)

                    def emit_cell(d, t):
                        t12[d] = rp.tile([8, 512], f32, tag=f"t12{d}", bufs=2, name=f"t12{d}")
                        if t == 0:
                            nc.vector.tensor_tensor(t12[d][:, 0:H], sif[d][:, 0:H], tgc[d][:, 0:H], op=OP.mult)
                            nc.vector.memset(t12[d][:, H:2 * H], 0.0)
                        else:
                            nc.vector.tensor_tensor(t12[d][:], sif[d][:], tgc[d][:], op=OP.mult)

                    def emit_transposes(d, t):
                        tp[d] = rpp.tile([128, 32], f32, tag=f"tp{d}", bufs=1, name=f"tp{d}")
                        for k in range(2):
                            nc.tensor.matmul(tp[d][:, k * 8:(k + 1) * 8],
                                             t12[d][:, k * 128:(k + 1) * 128], id8[:],
                                             is_transpose=True, start=True, stop=False)
                            nc.tensor.matmul(tp[d][:, k * 8:(k + 1) * 8],
                                             t12[d][:, H + k * 128:H + (k + 1) * 128], id8[:],
                                             is_transpose=True, start=False, stop=True)
                        for k in range(2):
                            nc.tensor.transpose(tp[d][:, 16 + k * 8:16 + (k + 1) * 8],
                                                so[d][:, k * 128:(k + 1) * 128], id8[:])

                    def emit_cadd(d, t):
                        nc.vector.tensor_tensor(tgc[d][:, H:2 * H], t12[d][:, 0:H],
                                                t12[d][:, H:2 * H], op=OP.add)

                    def emit_tail(d, t):
                        thT[d] = rp.tile([128, 16], f32, tag=f"thT{d}", bufs=2, name=f"thT{d}")
                        nc.scalar.activation(thT[d][:], tp[d][:, 0:16], AF.Tanh)
                        s, j = sj_of(t, d)
                        nc.vector.tensor_tensor(
                            xv[d][:, :, s, :],
                            thT[d][:].rearrange("p (k b) -> p k b", k=2),
                            tp[d][:, 16:32].rearrange("p (k b) -> p k b", k=2),
                            op=OP.mult,
                        )

                    for u in range(T_ + 1):
                        t0_, t1_ = u, u - 1
                        gb_next0 = gb_next1 = None
                        if t0_ < T_:
                            emit_rec_mms(0, t0_)
                            if t0_ + 1 < T_:
                                gb_next0 = emit_inject_d(0, t0_ + 1)
                        if t1_ >= 0:
                            if t1_ == 0:
                                gb[1] = emit_inject_d(1, 0)
                            emit_rec_mms(1, t1_)
                            if t1_ + 1 < T_:
                                gb_next1 = emit_inject_d(1, t1_ + 1)
                        if t0_ < T_:
                            emit_sig(0, t0_)
                            emit_cell(0, t0_)
                            emit_transposes(0, t0_)
                            emit_cadd(0, t0_)
                        if t1_ >= 0:
                            emit_sig(1, t1_)
                        if t0_ < T_:
                            emit_tail(0, t0_)
                        if t1_ >= 0:
                            emit_cell(1, t1_)
                            emit_transposes(1, t1_)
                            emit_cadd(1, t1_)
                            emit_tail(1, t1_)
                        if t0_ < T_:
                            gb[0] = gb_next0
                        if t1_ >= 0:
                            gb[1] = gb_next1

            # ================= layers =================
            with tc.tile_pool(name="x1p", bufs=1) as x1pool:
                x1T = {d: x1pool.tile([128, 2 * ntok], f32r, tag=f"x1T{d}", name=f"x1T{d}") for d in range(2)}
                with tc.tile_pool(name="x0p", bufs=1) as x0pool:
                    x0T = [x0pool.tile([128, ntok], f32r, tag=f"x0T{i}", name=f"x0T{i}") for i in range(2)]
                    for k in range(2):
                        nc.sync.dma_start(x0T[k][:], x0T_d[k * 128:(k + 1) * 128, :])
                    proj_phase(0, lambda k, lo, hi: x0T[k][:, lo:hi], 2)
                rec_phase(0, x1T)
                proj_phase(1, lambda k, lo, hi: x1T[k // 2][:, (k % 2) * ntok + lo:(k % 2) * ntok + hi], 4)
            with tc.tile_pool(name="x2p", bufs=1) as x2pool:
                x2T = {d: x2pool.tile([128, 2 * ntok], f32r, tag=f"x2T{d}", name=f"x2T{d}") for d in range(2)}
                rec_phase(1, x2T)
                # ---------- emissions ----------
                with (
                    tc.tile_pool(name="emis", bufs=2) as mp,
                    tc.tile_pool(name="emisps", bufs=2, space="PSUM") as mpp,
                ):
                    for w in range(ntok // 512):
                        ps = mpp.tile([K, 512], f32, tag="em")
                        for k in range(4 * NWO):
                            kk = k // NWO
                            nc.tensor.matmul(
                                ps[:], wout[k][:],
                                x2T[kk // 2][:, (kk % 2) * ntok + w * 512:(kk % 2) * ntok + (w + 1) * 512],
                                start=(k == 0), stop=(k == 4 * NWO - 1),
                            )
                        st = mp.tile([K, 512], f32, tag="emst")
                        nc.vector.tensor_scalar(st[:], ps[:], bout[:, 0:1], None, op0=OP.add)
                        nc.sync.dma_start(em_d[:, w * 512:(w + 1) * 512], st[:])
            # ================= viterbi forward =================
            with (
                tc.tile_pool(name="vit", bufs=1) as vp,
                tc.tile_pool(name="vitps", bufs=1, space="PSUM") as vpp,
            ):
                emP, score, bpf32 = {}, {}, {}
                for c in range(2):
                    emP[c] = vp.tile([128, T_], f32, tag=f"emP{c}", name=f"emP{c}")
                    nc.sync.dma_start(
                        emP[c][:],
                        AP(em_d, c * 4 * T_, [[T_, 4], [ntok, 32], [1, T_]]),
                    )
                    score[c] = vp.tile([128, 1], f32, tag=f"score{c}", name=f"score{c}")
                    nc.vector.tensor_tensor(score[c][:], srep[:], emP[c][:, 0:1], op=OP.add)
                    bpf32[c] = vp.tile([128, T_ - 1], f32r, tag=f"bpf{c}", name=f"bpf{c}")
                prev_m8 = {0: None, 1: None}
                for t in range(1, T_):
                    for c in range(2):
                        pre = vp.tile([128, K], f32, tag=f"pre{c}", bufs=2)
                        if t == 1:
                            nc.vector.tensor_scalar(pre[:], trep[:], score[c][:, 0:1], None, op0=OP.add)
                        else:
                            nc.vector.tensor_scalar(
                                pre[:], trep[:], prev_m8[c][:, 0:1],
                                emP[c][:, t - 1:t], op0=OP.add, op1=OP.add,
                            )
                        sT = vp.tile([128, K], f32, tag=f"sT{c}", bufs=2)
                        nc.vector.transpose(sT[:], pre[:])
                        m8 = vp.tile([128, 8], f32, tag=f"m8{c}", bufs=2)
                        nc.vector.max(m8[:], sT[:])
                        bp8 = vp.tile([128, 8], u16, tag=f"bp8{c}", bufs=2)
                        nc.vector.max_index(bp8[:], m8[:], sT[:])
                        nc.scalar.activation(bpf32[c][:, t - 1:t], bp8[:, 0:1], AF.Copy)
                        prev_m8[c] = m8
                # final score + last tag
                fi8, ltf = {}, {}
                for c in range(2):
                    nc.vector.scalar_tensor_tensor(
                        score[c][:], prev_m8[c][:, 0:1], emP[c][:, T_ - 1:T_],
                        erep[:], op0=OP.add, op1=OP.add,
                    )
                    fin = vp.tile([128, K], f32, tag=f"fin{c}")
                    nc.vector.tensor_copy(fin[:], score[c][:, 0:1].to_broadcast([128, K]))
                    finT = vp.tile([128, K], f32, tag=f"finT{c}")
                    nc.vector.transpose(finT[:], fin[:])
                    fm8 = vp.tile([128, 8], f32, tag=f"fm8{c}")
                    fi8[c] = vp.tile([128, 8], u32, tag=f"fi8{c}", name=f"fi8{c}")
                    nc.vector.max(fm8[:], finT[:])
                    nc.vector.max_index(fi8[c][:], fm8[:], finT[:])
                # ================= backtrace =================
                # cur kept as a one-hot column set [128,(i)]; per step ONE matmul
                # with broadcast bp-column as stationary computes
                # cb[p,i] = sum_q bp[q]*onehot[q,i] (= selected tag, bcast to all
                # partitions), then is_equal against kidx rebuilds the one-hot.
                tags1, onehot = {}, {}
                for c in range(2):
                    ltf[c] = vp.tile([128, 1], f32r, tag=f"ltf{c}", name=f"ltf{c}")
                    nc.scalar.activation(ltf[c][:], fi8[c][:, 0:1], AF.Copy)
                    onehot[c] = vp.tile([128, 4], f32r, tag=f"oh{c}", bufs=2, name=f"oh{c}")
                    tags1[c] = vp.tile([1, 4 * T_], f32r, tag=f"tg1{c}", name=f"tg1{c}")

                def step_sel(c, val_col, sel_ap, tcol):
                    cb = vpp.tile([128, 4], f32, tag=f"cb{c}", bufs=2, name=f"cb{c}")
                    nc.tensor.matmul(cb[:], val_col.to_broadcast([128, 128]), sel_ap,
                                     start=True, stop=True)
                    nc.vector.tensor_copy(tags1[c][0:1, tcol * 4:(tcol + 1) * 4], cb[0:1, :])
                    nc.vector.tensor_tensor(onehot[c][:], kidx[:], cb[:], op=OP.is_equal)

                for c in range(2):
                    step_sel(c, ltf[c][:, 0:1], sel4[:], T_ - 1)
                for t in range(T_ - 2, -1, -1):
                    for c in range(2):
                        step_sel(c, bpf32[c][:, t:t + 1], onehot[c][:], t)
                for c in range(2):
                    ti = vp.tile([1, 4 * T_], i32, tag=f"ti{c}", name=f"ti{c}")
                    nc.vector.tensor_copy(ti[:], tags1[c][:])
                    nc.sync.dma_start(tags_d[c][:], ti[:])
    nc.compile()
    return nc


def _prep_inputs(inputs, T_):
    """Host preprocessing -> per-core input maps."""
    d = {k: np.asarray(v) for k, v in inputs.items()}
    ids_full = d["inputs"].astype(np.int64)  # [B, T]
    emb = d["emb"].astype(np.float32)

    def stack_kchunks(w, nkc, comp):
        parts = []
        for k in range(nkc):
            chunk = w[k * 128:(k + 1) * 128, :]
            if comp:
                hi, lo = _split_hi_lo(chunk)
                parts += [hi, lo]
            else:
                parts += [chunk]
        return np.concatenate(parts, axis=0)

    def wih_pack(l):
        nkc = 2 if l == 0 else 4
        blocks = []
        for dr in ("f", "b"):
            w = _pack_gates_rows(d[f"W_ih_l{l}_{dr}"]).T.astype(np.float32)
            blocks.append(stack_kchunks(w, nkc, WCOMP_IH))
        return np.concatenate(blocks, axis=0)

    def whh_pack(l):
        blocks = []
        for dr in ("f", "b"):
            w = _pack_gates_rows(d[f"W_hh_l{l}_{dr}"]).T.astype(np.float32)
            blocks.append(stack_kchunks(w, 2, WCOMP_HH))
        return np.concatenate(blocks, axis=0)

    brow = np.zeros((1, 4 * G4), np.float32)
    for l in range(2):
        for di, dr in enumerate(("f", "b")):
            bb = _pack_gates_rows((d[f"b_ih_l{l}_{dr}"] + d[f"b_hh_l{l}_{dr}"]).reshape(4 * H, 1))[:, 0]
            brow[0, (l * 2 + di) * G4:(l * 2 + di + 1) * G4] = bb

    woutT = d["W_out"].T.astype(np.float32)  # [2H=512, K]
    wout_pack = stack_kchunks(woutT, 4, WCOMP_OUT)

    ssel = np.zeros((128, 128), np.float32)
    for b in range(8):
        for j in range(16):
            ssel[b * 16 + j, j * 8 + b] = 1.0
    ones = np.ones((1, 128), np.float32)
    trep = np.tile(d["trans"].astype(np.float32), (4, 1))
    srep = np.tile(d["start_trans"].astype(np.float32), 4).reshape(128, 1)
    erep = np.tile(d["end_trans"].astype(np.float32), 4).reshape(128, 1)
    kidx = np.full((128, 4), -1.0, np.float32)
    sel4 = np.zeros((128, 4), np.float32)
    for i in range(4):
        kidx[i * 32:(i + 1) * 32, i] = np.arange(32, dtype=np.float32)
        sel4[i * 32, i] = 1.0

    common = dict(
        wih0=wih_pack(0), wih1=wih_pack(1),
        whh0=whh_pack(0), whh1=whh_pack(1),
        brow=brow, wout=wout_pack,
        bout=d["b_out"].astype(np.float32).reshape(K, 1),
        ssel=ssel, ones=ones,
        trep=trep, srep=srep, erep=erep,
        kidx=kidx, sel4=sel4,
    )
    in_maps = []
    for core in range(NCORES):
        ids_core = ids_full[core * BS:(core + 1) * BS, :T_]  # [BS, T]
        x0 = emb[ids_core]  # [BS, T, E]
        x0T = np.ascontiguousarray(x0.reshape(BS * T_, E).T)  # [E, BS*T]
        m = dict(common)
        m["x0T"] = x0T
        in_maps.append(m)
    return in_maps


def _assemble(res, T_):
    tags = np.zeros((B, T_), np.int32)
    for core in range(NCORES):
        r = res[core]
        for c in range(2):
            tags[core * BS + c * 4:core * BS + (c + 1) * 4, :] = (
                r[f"tags{c}"].reshape(T_, 4).T)
    return tags


_staged = {}


def _get_staged(T_):
    """Jitted 8-core staged executable (cached per T_; avoids per-call jax retrace)."""
    if T_ in _staged:
        return _staged[T_]
    import jax
    from jax.sharding import Mesh, PartitionSpec, NamedSharding
    from jax.experimental.shard_map import shard_map
    from concourse import mybir
    from concourse.bass2jax import (
        _bass_exec_p, install_neuronx_cc_hook, partition_id_tensor,
    )

    if T_ not in _cache:
        _cache[T_] = _build(T_)
    nc = _cache[T_]
    install_neuronx_cc_hook()
    partition_name = nc.partition_id_tensor.name if nc.partition_id_tensor else None
    in_names, out_names, out_avals, zero_outs = [], [], [], []
    for alloc in nc.m.functions[0].allocations:
        if not isinstance(alloc, mybir.MemoryLocationSet):
            continue
        name = alloc.memorylocations[0].name
        if alloc.kind == "ExternalInput":
            if name != partition_name:
                in_names.append(name)
        elif alloc.kind == "ExternalOutput":
            shape = tuple(alloc.tensor_shape)
            dtype = mybir.dt.np(alloc.dtype)
            out_names.append(name)
            out_avals.append(jax.core.ShapedArray(shape, dtype))
            zero_outs.append(np.zeros(shape, dtype))
    all_names = list(in_names) + list(out_names)
    if partition_name is not None:
        all_names.append(partition_name)

    def _body(*args):
        operands = list(args)
        if partition_name is not None:
            operands.append(partition_id_tensor())
        return tuple(_bass_exec_p.bind(
            *operands, out_avals=tuple(out_avals), in_names=tuple(all_names),
            out_names=tuple(out_names), lowering_input_output_aliases=(),
            sim_require_finite=True, sim_require_nnan=True, nc=nc))

    devices = jax.devices()[:NCORES]
    mesh = Mesh(np.asarray(devices), ("core",))
    nio = len(in_names) + len(out_names)
    fn = jax.jit(
        shard_map(_body, mesh=mesh,
                  in_specs=(PartitionSpec("core"),) * nio,
                  out_specs=(PartitionSpec("core"),) * len(out_names),
                  check_rep=False),
        keep_unused=True,
    )
    sh = NamedSharding(mesh, PartitionSpec("core"))
    st = dict(fn=fn, sh=sh, in_names=in_names, out_names=out_names,
              zero_outs=zero_outs, jax=jax)
    _staged[T_] = st
    return st


def run(inputs, T_=T_FULL, trace=False):
    if trace:
        from concourse.bass_utils import run_bass_kernel_spmd

        if T_ not in _cache:
            _cache[T_] = _build(T_)
        in_maps = _prep_inputs(inputs, T_)
        res = run_bass_kernel_spmd(_cache[T_], in_maps, core_ids=list(range(NCORES)), trace=trace)
        return _assemble_maps(res.results, T_), res

    st = _get_staged(T_)
    jax = st["jax"]
    in_maps = _prep_inputs(inputs, T_)
    dev_in = [
        jax.device_put(
            np.concatenate([np.asarray(in_maps[c][n]) for c in range(NCORES)], axis=0),
            st["sh"])
        for n in st["in_names"]
    ]
    dev_zero = [
        jax.device_put(np.zeros((NCORES * z.shape[0], *z.shape[1:]), z.dtype), st["sh"])
        for z in st["zero_outs"]
    ]
    outs = st["fn"](*dev_in, *dev_zero)
    res = [
        {name: np.asarray(outs[i]).reshape(NCORES, *st["zero_outs"][i].shape)[c]
         for i, name in enumerate(st["out_names"])}
        for c in range(NCORES)
    ]
    return _assemble_maps(res, T_), None


def _assemble_maps(res, T_):
    return _assemble(res, T_)


def kernel(**inputs):
    tags, _ = run(inputs)
    return tags


# revision 5
# speedup vs baseline: 1.0257x; 1.0257x over previous
"""BiLSTM-CRF on 8 Trainium2 NeuronCores — batch-data-parallel, v2.

Contract: kernel(**inputs) takes the FULL unsharded inputs (as produced by
setup_inputs) and returns the full [B, T] int32 tag tensor.

v2 changes vs baseline:
  - embedding gather + input transpose moved to host (x0T shipped per core)
  - optional hi/lo weight compensation per weight group (flags below)
  - recurrence: per-bank PSUM accumulation (sigmoid starts after half the
    matmuls), c/sigma(o) transposed instead of h (shorter critical chain,
    single fused write of hT into the next layer's input tile)
  - Viterbi backtrace on device via one-hot selection matmuls; outputs are
    the final [4, T] tag rows per chain (130x smaller transfers)
"""

import numpy as np

B, T_FULL, V, E, H, K = 64, 512, 30000, 256, 256, 32
NCORES = 8
BS = B // NCORES
G4 = 4 * H

WCOMP_IH = False  # host hi/lo split of W_ih (proj weights)
WCOMP_HH = False  # host hi/lo split of W_hh (recurrent weights)
WCOMP_OUT = True  # host hi/lo split of W_out (emissions; cheap, keep)

_cache = {}


def _split_hi_lo(w):
    w = np.asarray(w, np.float32)
    m, e = np.frexp(w)
    hi = np.ldexp(np.round(m * 4096.0) / 4096.0, e).astype(np.float32)
    lo = (w.astype(np.float64) - hi).astype(np.float32)
    return hi, lo


def _pack_gates_rows(w):
    """Reorder leading 4H axis from (i,f,g,o) to (i,f,o,g)."""
    i, f, g, o = np.split(np.asarray(w), 4, axis=0)
    return np.concatenate([i, f, o, g], axis=0)


def _build(T_):
    import concourse.bass as bass
    import concourse.tile as tile
    from concourse import bacc, mybir
    from concourse.masks import make_identity

    f32 = mybir.dt.float32
    f32r = mybir.dt.float32r
    u16 = mybir.dt.uint16
    u32 = mybir.dt.uint32
    i32 = mybir.dt.int32
    AF = mybir.ActivationFunctionType
    OP = mybir.AluOpType
    AP = bass.AP

    ntok = BS * T_
    nw128 = ntok // 128
    NWI = 2 if WCOMP_IH else 1
    NWH = 2 if WCOMP_HH else 1
    NWO = 2 if WCOMP_OUT else 1

    nc = bacc.Bacc(None, target_bir_lowering=False)

    x0T_d = nc.dram_tensor("x0T", [2 * 128, ntok], f32r, kind="ExternalInput")
    wih0_d = nc.dram_tensor("wih0", [2 * 2 * NWI * 128, G4], f32r, kind="ExternalInput")
    wih1_d = nc.dram_tensor("wih1", [2 * 4 * NWI * 128, G4], f32r, kind="ExternalInput")
    whh0_d = nc.dram_tensor("whh0", [2 * 2 * NWH * 128, G4], f32r, kind="ExternalInput")
    whh1_d = nc.dram_tensor("whh1", [2 * 2 * NWH * 128, G4], f32r, kind="ExternalInput")
    brow_d = nc.dram_tensor("brow", [1, 4 * G4], f32r, kind="ExternalInput")
    wout_d = nc.dram_tensor("wout", [4 * NWO * 128, K], f32r, kind="ExternalInput")
    bout_d = nc.dram_tensor("bout", [K, 1], f32, kind="ExternalInput")
    ssel_d = nc.dram_tensor("ssel", [128, 128], f32r, kind="ExternalInput")
    ones_d = nc.dram_tensor("ones", [1, 128], f32r, kind="ExternalInput")
    trep_d = nc.dram_tensor("trep", [128, K], f32, kind="ExternalInput")
    srep_d = nc.dram_tensor("srep", [128, 1], f32, kind="ExternalInput")
    erep_d = nc.dram_tensor("erep", [128, 1], f32, kind="ExternalInput")
    kidx_d = nc.dram_tensor("kidx", [128, 4], f32r, kind="ExternalInput")
    sel4_d = nc.dram_tensor("sel4", [128, 4], f32r, kind="ExternalInput")

    tags_d = [nc.dram_tensor(f"tags{c}", [1, 4 * T_], i32, kind="ExternalOutput") for c in range(2)]

    gx_d = {(l, d): nc.dram_tensor(f"gx{l}{d}", [ntok, G4], f32r, kind="Internal")
            for l in range(2) for d in range(2)}
    em_d = nc.dram_tensor("em_scr", [K, ntok], f32, kind="Internal")

    with tile.TileContext(nc) as tc:
        with (
            tc.tile_pool(name="const", bufs=1) as cpool,
        ):
            whh = {}
            for l, dram in [(0, whh0_d), (1, whh1_d)]:
                for d in range(2):
                    for k in range(2 * NWH):
                        t_ = cpool.tile([128, G4], f32r, tag=f"whh{l}{d}{k}", name=f"whh{l}{d}{k}")
                        nc.sync.dma_start(t_[:], dram[(d * 2 * NWH + k) * 128:(d * 2 * NWH + k + 1) * 128, :])
                        whh[(l, d, k)] = t_
            ssel = cpool.tile([128, 128], f32r, tag="ssel")
            nc.sync.dma_start(ssel[:], ssel_d[:])
            brow = cpool.tile([1, 4 * G4], f32r, tag="brow")
            nc.sync.dma_start(brow[:], brow_d[:])
            ones1 = cpool.tile([1, 128], f32r, tag="ones1")
            nc.sync.dma_start(ones1[:], ones_d[:])
            id8 = cpool.tile([8, 8], f32, tag="id8")
            make_identity(nc, id8[:])
            wout = {}
            for k in range(4 * NWO):
                t_ = cpool.tile([128, K], f32r, tag=f"wout{k}", name=f"wout{k}")
                nc.sync.dma_start(t_[:], wout_d[k * 128:(k + 1) * 128, :])
                wout[k] = t_
            bout = cpool.tile([K, 1], f32, tag="bout")
            nc.sync.dma_start(bout[:], bout_d[:])
            trep = cpool.tile([128, K], f32, tag="trep")
            nc.sync.dma_start(trep[:], trep_d[:])
            srep = cpool.tile([128, 1], f32, tag="srep")
            nc.sync.dma_start(srep[:], srep_d[:])
            erep = cpool.tile([128, 1], f32, tag="erep")
            nc.sync.dma_start(erep[:], erep_d[:])
            kidx = cpool.tile([128, 4], f32r, tag="kidx")
            nc.sync.dma_start(kidx[:], kidx_d[:])
            sel4 = cpool.tile([128, 4], f32r, tag="sel4")
            nc.sync.dma_start(sel4[:], sel4_d[:])

            def proj_phase(l, xT_view, nkc):
                """xT_view(k, lo, hi) -> AP [128, hi-lo] of input chunk k."""
                wih_dram = wih0_d if l == 0 else wih1_d
                with (
                    tc.tile_pool(name=f"proj{l}", bufs=1) as pp,
                    tc.tile_pool(name=f"projps{l}", bufs=2, space="PSUM") as ppp,
                ):
                    for d in range(2):
                        wih = {}
                        for k in range(nkc * NWI):
                            t_ = pp.tile([128, G4], f32r, tag=f"wih{k}", name=f"wih{k}")
                            nc.sync.dma_start(
                                t_[:],
                                wih_dram[(d * nkc * NWI + k) * 128:(d * nkc * NWI + k + 1) * 128, :],
                            )
                            wih[k] = t_
                        for w in range(nw128):
                            ps = ppp.tile([128, G4], f32, tag="pj")
                            for nh in range(2):
                                sl = slice(nh * 512, (nh + 1) * 512)
                                nc.tensor.matmul(
                                    ps[:, sl], ones1[:, :],
                                    brow[0:1, (l * 2 + d) * G4 + nh * 512:(l * 2 + d) * G4 + (nh + 1) * 512],
                                    start=True, stop=False,
                                )
                                for k in range(nkc * NWI):
                                    nc.tensor.matmul(
                                        ps[:, sl],
                                        xT_view(k // NWI, w * 128, (w + 1) * 128),
                                        wih[k][:, sl],
                                        start=False, stop=(k == nkc * NWI - 1),
                                    )
                            st = pp.tile([128, G4], f32r, tag="pjst", bufs=2)
                            nc.vector.tensor_copy(st[:], ps[:])
                            nc.sync.dma_start(gx_d[(l, d)][w * 128:(w + 1) * 128, :], st[:])

            def rec_phase(l, xnext):
                """xnext: dict d -> tile [128, 2*ntok] (k-chunk major)."""
                with (
                    tc.tile_pool(name=f"rec{l}", bufs=1) as rp,
                    tc.tile_pool(name=f"recps{l}", bufs=1, space="PSUM") as rpp,
                ):
                    # tgc: [tanh(g) | c] adjacent so t1|t2 is ONE tensor_tensor
                    tgc = {d: rp.tile([8, 512], f32, tag=f"tgc{d}", name=f"tgc{d}") for d in range(2)}
                    nwin = T_ // 16

                    def win_dma(d, wi):
                        wt = rp.tile([128, G4], f32r, tag=f"win{d}", bufs=2, name=f"win{d}")
                        nc.sync.dma_start(
                            wt[:],
                            AP(gx_d[(l, d)], wi * 16 * G4,
                               [[T_ * G4, BS], [G4, 16], [1, G4]]),
                        )
                        return wt

                    win = {0: win_dma(0, 0), 1: win_dma(1, nwin - 1)}
                    win_next = {}
                    xv = {d: xnext[d][:].rearrange("p (k b t) -> p k t b", k=2, b=BS)
                          for d in range(2)}

                    def sj_of(t, d):
                        s = t if d == 0 else T_ - 1 - t
                        return s, s % 16

                    def emit_inject_d(d, t):
                        """Window bookkeeping + gx-inject matmuls for (d, t)."""
                        s, j = sj_of(t, d)
                        wi = s // 16
                        if t > 0 and j == (0 if d == 0 else 15):
                            win[d] = win_next[d]
                        if j == (8 if d == 0 else 7):
                            nwi = wi + 1 if d == 0 else wi - 1
                            if 0 <= nwi < nwin:
                                win_next[d] = win_dma(d, nwi)
                        gbs = {}
                        for nh in range(2):
                            g_ps = rpp.tile([8, 512], f32, tag=f"g{d}{nh}",
                                            bufs=(2 if d == 0 else 1), name=f"g{d}{nh}")
                            nc.tensor.matmul(
                                g_ps[:], ssel[:, j * 8:(j + 1) * 8],
                                win[d][:, nh * 512:(nh + 1) * 512],
                                start=True, stop=(t == 0),
                            )
                            gbs[nh] = g_ps
                        return gbs

                    # The two direction-chains run skewed by one step: while d0
                    # is at step t, d1 is at step t-1, so each dir's ACT block
                    # fills the other's non-ACT spine segments.
                    gb = {0: emit_inject_d(0, 0), 1: None}
                    sif = [None, None]
                    so = [None, None]
                    t12 = [None, None]
                    tp = [None, None]
                    thT = [None, None]

                    def emit_rec_mms(d, t):
                        if t == 0:
                            return
                        s, j = sj_of(t, d)
                        sp = s - 1 if d == 0 else s + 1
                        for nh in range(2):
                            for k in range(2 * NWH):
                                nc.tensor.matmul(
                                    gb[d][nh][:],
                                    xv[d][:, k // NWH, sp, :],
                                    whh[(l, d, k)][:, nh * 512:(nh + 1) * 512],
                                    start=False,
                                    stop=(k == 2 * NWH - 1),
                                )

                    def emit_sig(d, t):
                        sif[d] = rp.tile([8, 512], f32, tag=f"sif{d}", bufs=2, name=f"sif{d}")
                        nc.scalar.activation(sif[d][:], gb[d][0][:], AF.Sigmoid)
                        nc.scalar.activation(tgc[d][:, 0:H], gb[d][1][:, H:2 * H], AF.Tanh)
                        so[d] = rp.tile([8, H], f32, tag=f"so{d}", bufs=2, name=f"so{d}")
                        nc.scalar.activation(so[d][:], gb[d][1][:, 0:H], AF.Sigmoid)

                    def emit_cell(d, t):
                        t12[d] = rp.tile([8, 512], f32, tag=f"t12{d}", bufs=2, name=f"t12{d}")
                        if t == 0:
                            nc.vector.tensor_tensor(t12[d][:, 0:H], sif[d][:, 0:H], tgc[d][:, 0:H], op=OP.mult)
                            nc.vector.memset(t12[d][:, H:2 * H], 0.0)
                        else:
                            nc.vector.tensor_tensor(t12[d][:], sif[d][:], tgc[d][:], op=OP.mult)

                    def emit_transposes(d, t):
                        tp[d] = rpp.tile([128, 32], f32, tag=f"tp{d}", bufs=1, name=f"tp{d}")
                        for k in range(2):
                            nc.tensor.matmul(tp[d][:, k * 8:(k + 1) * 8],
                                             t12[d][:, k * 128:(k + 1) * 128], id8[:],
                                             is_transpose=True, start=True, stop=False)
                            nc.tensor.matmul(tp[d][:, k * 8:(k + 1) * 8],
                                             t12[d][:, H + k * 128:H + (k + 1) * 128], id8[:],
                                             is_transpose=True, start=False, stop=True)
                        for k in range(2):
                            nc.tensor.transpose(tp[d][:, 16 + k * 8:16 + (k + 1) * 8],
                                                so[d][:, k * 128:(k + 1) * 128], id8[:])

                    def emit_cadd(d, t):
                        nc.vector.tensor_tensor(tgc[d][:, H:2 * H], t12[d][:, 0:H],
                                                t12[d][:, H:2 * H], op=OP.add)

                    def emit_tail(d, t):
                        thT[d] = rp.tile([128, 16], f32, tag=f"thT{d}", bufs=2, name=f"thT{d}")
                        nc.scalar.activation(thT[d][:], tp[d][:, 0:16], AF.Tanh)
                        s, j = sj_of(t, d)
                        nc.vector.tensor_tensor(
                            xv[d][:, :, s, :],
                            thT[d][:].rearrange("p (k b) -> p k b", k=2),
                            tp[d][:, 16:32].rearrange("p (k b) -> p k b", k=2),
                            op=OP.mult,
                        )

                    for u in range(T_ + 1):
                        t0_, t1_ = u, u - 1
                        gb_next0 = gb_next1 = None
                        if t0_ < T_:
                            emit_rec_mms(0, t0_)
                            if t0_ + 1 < T_:
                                gb_next0 = emit_inject_d(0, t0_ + 1)
                        if t1_ >= 0:
                            if t1_ == 0:
                                gb[1] = emit_inject_d(1, 0)
                            emit_rec_mms(1, t1_)
                            if t1_ + 1 < T_:
                                gb_next1 = emit_inject_d(1, t1_ + 1)
                        if t0_ < T_:
                            emit_sig(0, t0_)
                            emit_cell(0, t0_)
                            emit_transposes(0, t0_)
                            emit_cadd(0, t0_)
                        if t1_ >= 0:
                            emit_sig(1, t1_)
                        if t0_ < T_:
                            emit_tail(0, t0_)
                        if t1_ >= 0:
                            emit_cell(1, t1_)
                            emit_transposes(1, t1_)
                            emit_cadd(1, t1_)
                            emit_tail(1, t1_)
                        if t0_ < T_:
                            gb[0] = gb_next0
                        if t1_ >= 0:
                            gb[1] = gb_next1

            # ================= layers =================
            with tc.tile_pool(name="x1p", bufs=1) as x1pool:
                x1T = {d: x1pool.tile([128, 2 * ntok], f32r, tag=f"x1T{d}", name=f"x1T{d}") for d in range(2)}
                with tc.tile_pool(name="x0p", bufs=1) as x0pool:
                    x0T = [x0pool.tile([128, ntok], f32r, tag=f"x0T{i}", name=f"x0T{i}") for i in range(2)]
                    for k in range(2):
                        nc.sync.dma_start(x0T[k][:], x0T_d[k * 128:(k + 1) * 128, :])
                    proj_phase(0, lambda k, lo, hi: x0T[k][:, lo:hi], 2)
                rec_phase(0, x1T)
                proj_phase(1, lambda k, lo, hi: x1T[k // 2][:, (k % 2) * ntok + lo:(k % 2) * ntok + hi], 4)
            with tc.tile_pool(name="x2p", bufs=1) as x2pool:
                x2T = {d: x2pool.tile([128, 2 * ntok], f32r, tag=f"x2T{d}", name=f"x2T{d}") for d in range(2)}
                rec_phase(1, x2T)
                # ---------- emissions ----------
                with (
                    tc.tile_pool(name="emis", bufs=2) as mp,
                    tc.tile_pool(name="emisps", bufs=2, space="PSUM") as mpp,
                ):
                    for w in range(ntok // 512):
                        ps = mpp.tile([K, 512], f32, tag="em")
                        for k in range(4 * NWO):
                            kk = k // NWO
                            nc.tensor.matmul(
                                ps[:], wout[k][:],
                                x2T[kk // 2][:, (kk % 2) * ntok + w * 512:(kk % 2) * ntok + (w + 1) * 512],
                                start=(k == 0), stop=(k == 4 * NWO - 1),
                            )
                        st = mp.tile([K, 512], f32, tag="emst")
                        nc.vector.tensor_scalar(st[:], ps[:], bout[:, 0:1], None, op0=OP.add)
                        nc.sync.dma_start(em_d[:, w * 512:(w + 1) * 512], st[:])
            # ================= viterbi forward =================
            with (
                tc.tile_pool(name="vit", bufs=1) as vp,
                tc.tile_pool(name="vitps", bufs=1, space="PSUM") as vpp,
            ):
                emP, score, bpf32 = {}, {}, {}
                for c in range(2):
                    emP[c] = vp.tile([128, T_], f32, tag=f"emP{c}", name=f"emP{c}")
                    nc.sync.dma_start(
                        emP[c][:],
                        AP(em_d, c * 4 * T_, [[T_, 4], [ntok, 32], [1, T_]]),
                    )
                    score[c] = vp.tile([128, 1], f32, tag=f"score{c}", name=f"score{c}")
                    nc.vector.tensor_tensor(score[c][:], srep[:], emP[c][:, 0:1], op=OP.add)
                    bpf32[c] = vp.tile([128, T_ - 1], f32r, tag=f"bpf{c}", name=f"bpf{c}")
                prev_m8 = {0: None, 1: None}
                for t in range(1, T_):
                    for c in range(2):
                        pre = vp.tile([128, K], f32, tag=f"pre{c}", bufs=2)
                        if t == 1:
                            nc.vector.tensor_scalar(pre[:], trep[:], score[c][:, 0:1], None, op0=OP.add)
                        else:
                            nc.vector.tensor_scalar(
                                pre[:], trep[:], prev_m8[c][:, 0:1],
                                emP[c][:, t - 1:t], op0=OP.add, op1=OP.add,
                            )
                        sT = vp.tile([128, K], f32, tag=f"sT{c}", bufs=2)
                        nc.vector.transpose(sT[:], pre[:])
                        m8 = vp.tile([128, 8], f32, tag=f"m8{c}", bufs=2)
                        nc.vector.max(m8[:], sT[:])
                        bp8 = vp.tile([128, 8], u16, tag=f"bp8{c}", bufs=2)
                        nc.vector.max_index(bp8[:], m8[:], sT[:])
                        nc.scalar.activation(bpf32[c][:, t - 1:t], bp8[:, 0:1], AF.Copy)
                        prev_m8[c] = m8
                # final score + last tag
                fi8, ltf = {}, {}
                for c in range(2):
                    nc.vector.scalar_tensor_tensor(
                        score[c][:], prev_m8[c][:, 0:1], emP[c][:, T_ - 1:T_],
                        erep[:], op0=OP.add, op1=OP.add,
                    )
                    fin = vp.tile([128, K], f32, tag=f"fin{c}")
                    nc.vector.tensor_copy(fin[:], score[c][:, 0:1].to_broadcast([128, K]))
                    finT = vp.tile([128, K], f32, tag=f"finT{c}")
                    nc.vector.transpose(finT[:], fin[:])
                    fm8 = vp.tile([128, 8], f32, tag=f"fm8{c}")
                    fi8[c] = vp.tile([128, 8], u32, tag=f"fi8{c}", name=f"fi8{c}")
                    nc.vector.max(fm8[:], finT[:])
                    nc.vector.max_index(fi8[c][:], fm8[:], finT[:])
                # ================= backtrace =================
                # cur kept as a one-hot column set [128,(i)]; per step ONE matmul
                # with broadcast bp-column as stationary computes
                # cb[p,i] = sum_q bp[q]*onehot[q,i] (= selected tag, bcast to all
                # partitions), then is_equal against kidx rebuilds the one-hot.
                tags1, onehot = {}, {}
                for c in range(2):
                    ltf[c] = vp.tile([128, 1], f32r, tag=f"ltf{c}", name=f"ltf{c}")
                    nc.scalar.activation(ltf[c][:], fi8[c][:, 0:1], AF.Copy)
                    onehot[c] = vp.tile([128, 4], f32r, tag=f"oh{c}", bufs=2, name=f"oh{c}")
                    tags1[c] = vp.tile([1, 4 * T_], f32r, tag=f"tg1{c}", name=f"tg1{c}")

                def step_sel(c, val_col, sel_ap, tcol):
                    cb = vpp.tile([128, 4], f32, tag=f"cb{c}", bufs=2, name=f"cb{c}")
                    nc.tensor.matmul(cb[:], val_col.to_broadcast([128, 128]), sel_ap,
                                     start=True, stop=True)
                    # is_equal first: it feeds the next selection matmul (the
                    # chain); the tags extraction is off-chain and runs on the
                    # otherwise-idle ACT engine
                    nc.vector.tensor_tensor(onehot[c][:], kidx[:], cb[:], op=OP.is_equal)
                    nc.scalar.activation(tags1[c][0:1, tcol * 4:(tcol + 1) * 4], cb[0:1, :], AF.Copy)

                for c in range(2):
                    step_sel(c, ltf[c][:, 0:1], sel4[:], T_ - 1)
                for t in range(T_ - 2, -1, -1):
                    for c in range(2):
                        step_sel(c, bpf32[c][:, t:t + 1], onehot[c][:], t)
                for c in range(2):
                    ti = vp.tile([1, 4 * T_], i32, tag=f"ti{c}", name=f"ti{c}")
                    nc.vector.tensor_copy(ti[:], tags1[c][:])
                    nc.sync.dma_start(tags_d[c][:], ti[:])
    nc.compile()
    return nc


def _prep_inputs(inputs, T_):
    """Host preprocessing -> per-core input maps."""
    d = {k: np.asarray(v) for k, v in inputs.items()}
    ids_full = d["inputs"].astype(np.int64)  # [B, T]
    emb = d["emb"].astype(np.float32)

    def stack_kchunks(w, nkc, comp):
        parts = []
        for k in range(nkc):
            chunk = w[k * 128:(k + 1) * 128, :]
            if comp:
                hi, lo = _split_hi_lo(chunk)
                parts += [hi, lo]
            else:
                parts += [chunk]
        return np.concatenate(parts, axis=0)

    def wih_pack(l):
        nkc = 2 if l == 0 else 4
        blocks = []
        for dr in ("f", "b"):
            w = _pack_gates_rows(d[f"W_ih_l{l}_{dr}"]).T.astype(np.float32)
            blocks.append(stack_kchunks(w, nkc, WCOMP_IH))
        return np.concatenate(blocks, axis=0)

    def whh_pack(l):
        blocks = []
        for dr in ("f", "b"):
            w = _pack_gates_rows(d[f"W_hh_l{l}_{dr}"]).T.astype(np.float32)
            blocks.append(stack_kchunks(w, 2, WCOMP_HH))
        return np.concatenate(blocks, axis=0)

    brow = np.zeros((1, 4 * G4), np.float32)
    for l in range(2):
        for di, dr in enumerate(("f", "b")):
            bb = _pack_gates_rows((d[f"b_ih_l{l}_{dr}"] + d[f"b_hh_l{l}_{dr}"]).reshape(4 * H, 1))[:, 0]
            brow[0, (l * 2 + di) * G4:(l * 2 + di + 1) * G4] = bb

    woutT = d["W_out"].T.astype(np.float32)  # [2H=512, K]
    wout_pack = stack_kchunks(woutT, 4, WCOMP_OUT)

    ssel = np.zeros((128, 128), np.float32)
    for b in range(8):
        for j in range(16):
            ssel[b * 16 + j, j * 8 + b] = 1.0
    ones = np.ones((1, 128), np.float32)
    trep = np.tile(d["trans"].astype(np.float32), (4, 1))
    srep = np.tile(d["start_trans"].astype(np.float32), 4).reshape(128, 1)
    erep = np.tile(d["end_trans"].astype(np.float32), 4).reshape(128, 1)
    kidx = np.full((128, 4), -1.0, np.float32)
    sel4 = np.zeros((128, 4), np.float32)
    for i in range(4):
        kidx[i * 32:(i + 1) * 32, i] = np.arange(32, dtype=np.float32)
        sel4[i * 32, i] = 1.0

    common = dict(
        wih0=wih_pack(0), wih1=wih_pack(1),
        whh0=whh_pack(0), whh1=whh_pack(1),
        brow=brow, wout=wout_pack,
        bout=d["b_out"].astype(np.float32).reshape(K, 1),
        ssel=ssel, ones=ones,
        trep=trep, srep=srep, erep=erep,
        kidx=kidx, sel4=sel4,
    )
    in_maps = []
    for core in range(NCORES):
        ids_core = ids_full[core * BS:(core + 1) * BS, :T_]  # [BS, T]
        x0 = emb[ids_core]  # [BS, T, E]
        x0T = np.ascontiguousarray(x0.reshape(BS * T_, E).T)  # [E, BS*T]
        m = dict(common)
        m["x0T"] = x0T
        in_maps.append(m)
    return in_maps


def _assemble(res, T_):
    tags = np.zeros((B, T_), np.int32)
    for core in range(NCORES):
        r = res[core]
        for c in range(2):
            tags[core * BS + c * 4:core * BS + (c + 1) * 4, :] = (
                r[f"tags{c}"].reshape(T_, 4).T)
    return tags


_staged = {}


def _get_staged(T_):
    """Jitted 8-core staged executable (cached per T_; avoids per-call jax retrace)."""
    if T_ in _staged:
        return _staged[T_]
    import jax
    from jax.sharding import Mesh, PartitionSpec, NamedSharding
    from jax.experimental.shard_map import shard_map
    from concourse import mybir
    from concourse.bass2jax import (
        _bass_exec_p, install_neuronx_cc_hook, partition_id_tensor,
    )

    if T_ not in _cache:
        _cache[T_] = _build(T_)
    nc = _cache[T_]
    install_neuronx_cc_hook()
    partition_name = nc.partition_id_tensor.name if nc.partition_id_tensor else None
    in_names, out_names, out_avals, zero_outs = [], [], [], []
    for alloc in nc.m.functions[0].allocations:
        if not isinstance(alloc, mybir.MemoryLocationSet):
            continue
        name = alloc.memorylocations[0].name
        if alloc.kind == "ExternalInput":
            if name != partition_name:
                in_names.append(name)
        elif alloc.kind == "ExternalOutput":
            shape = tuple(alloc.tensor_shape)
            dtype = mybir.dt.np(alloc.dtype)
            out_names.append(name)
            out_avals.append(jax.core.ShapedArray(shape, dtype))
            zero_outs.append(np.zeros(shape, dtype))
    all_names = list(in_names) + list(out_names)
    if partition_name is not None:
        all_names.append(partition_name)

    def _body(*args):
        operands = list(args)
        if partition_name is not None:
            operands.append(partition_id_tensor())
        return tuple(_bass_exec_p.bind(
            *operands, out_avals=tuple(out_avals), in_names=tuple(all_names),
            out_names=tuple(out_names), lowering_input_output_aliases=(),
            sim_require_finite=True, sim_require_nnan=True, nc=nc))

    devices = jax.devices()[:NCORES]
    mesh = Mesh(np.asarray(devices), ("core",))
    nio = len(in_names) + len(out_names)
    fn = jax.jit(
        shard_map(_body, mesh=mesh,
                  in_specs=(PartitionSpec("core"),) * nio,
                  out_specs=(PartitionSpec("core"),) * len(out_names),
                  check_rep=False),
        keep_unused=True,
    )
    sh = NamedSharding(mesh, PartitionSpec("core"))
    st = dict(fn=fn, sh=sh, in_names=in_names, out_names=out_names,
              zero_outs=zero_outs, jax=jax)
    _staged[T_] = st
    return st


def run(inputs, T_=T_FULL, trace=False):
    if trace:
        from concourse.bass_utils import run_bass_kernel_spmd

        if T_ not in _cache:
            _cache[T_] = _build(T_)
        in_maps = _prep_inputs(inputs, T_)
        res = run_bass_kernel_spmd(_cache[T_], in_maps, core_ids=list(range(NCORES)), trace=trace)
        return _assemble_maps(res.results, T_), res

    st = _get_staged(T_)
    jax = st["jax"]
    in_maps = _prep_inputs(inputs, T_)
    dev_in = [
        jax.device_put(
            np.concatenate([np.asarray(in_maps[c][n]) for c in range(NCORES)], axis=0),
            st["sh"])
        for n in st["in_names"]
    ]
    dev_zero = [
        jax.device_put(np.zeros((NCORES * z.shape[0], *z.shape[1:]), z.dtype), st["sh"])
        for z in st["zero_outs"]
    ]
    outs = st["fn"](*dev_in, *dev_zero)
    res = [
        {name: np.asarray(outs[i]).reshape(NCORES, *st["zero_outs"][i].shape)[c]
         for i, name in enumerate(st["out_names"])}
        for c in range(NCORES)
    ]
    return _assemble_maps(res, T_), None


def _assemble_maps(res, T_):
    return _assemble(res, T_)


def kernel(**inputs):
    tags, _ = run(inputs)
    return tags
